# revision 1
# baseline (speedup 1.0000x reference)
"""Trainium2 Bass kernel for nn_CrossAttention3D (sparse_attention).

Sharding (8 NeuronCores, zero collectives): core c handles batch b=c//4,
query rows [g*512, (g+1)*512), g=c%4. The k-projection is recomputed per
core (cheaper than an all-gather of k at this size).

Host-side prep (free): query/key pre-transposed to feature-major, norm
weights folded into Wq/Wk, 1/sqrt(DH) folded into the k side, the 1/H
head-mean folded into value, biases passed as rows.

Per-core pipeline (fp32 math, matmuls in float32r):
  R: r = rsqrt(mean(x^2)+eps) per token via exp(-0.5*ln(.)) (exact; stays
     in the natural_log_exp ACT table set shared with the softmax exp).
     sum(x^2) over features via ones-vector matmuls (features live on
     partitions); r broadcast back over partitions via a K=1 ones-outer.
  P: x-tiles scaled by r in place (GPSIMD), projections accumulate in
     PSUM over 8 feature chunks, bias added with a K=1 ones matmul,
     plain DVE copy to SBUF -> qS [1024d, 512i], kS [1024d, 2048j].
  S: per head h / 128-row i-tile: scores = qS_h^T @ kS_h into a PSUM
     [128, 2048] tile; single ACT exp with accum_out gives e and the
     softmax denominator; attn_it += e * (1/denom) split column-wise
     between DVE and GPSIMD (separate attn tiles so the two chains are
     independent). The k-projection (per d-chunk m) and S for heads
     2m, 2m+1 time-share one 8-bank PSUM pool, so PE/ACT/DVE/GPSIMD all
     stream concurrently.
  T: attn transposed with PE identity-matmuls (f32r transpose).
  F: out = attnT^T @ (value/16) accumulated over j in PSUM.
"""

import sys

sys.path.insert(0, "/opt/trn_rl_repo")

import numpy as np

import concourse.bass as bass
import concourse.tile as tile
from concourse import bacc
from concourse import mybir
from concourse.bass_utils import run_bass_kernel_spmd
from concourse.masks import make_identity

B, Q, KV, D, H, DH = 2, 2048, 2048, 1024, 16, 64
QC = 512
P = 128
EPS = float(np.finfo(np.float32).eps)
F32 = mybir.dt.float32
F32R = mybir.dt.float32r
AF = mybir.ActivationFunctionType
ALU = mybir.AluOpType

DVE_COLS = 1152  # attn combine: columns handled by DVE; rest on GPSIMD
GP_COLS = KV - DVE_COLS

_cache = {}
PHASES = "full"  # debug knob: pq | rk | pk | s_nocomb | s | tf=full
TRACE_SIM = False


def build_nc() -> bass.Bass:
    nc = bacc.Bacc()

    qT = nc.declare_dram_parameter("qT", [D, QC], F32R, isOutput=False)
    kT = nc.declare_dram_parameter("kT", [D, KV], F32R, isOutput=False)
    v = nc.declare_dram_parameter("v", [KV, D], F32R, isOutput=False)
    wqT = nc.declare_dram_parameter("wqT", [D, D], F32R, isOutput=False)
    wkT = nc.declare_dram_parameter("wkT", [D, D], F32R, isOutput=False)
    bqp = nc.declare_dram_parameter("bq", [D], F32R, isOutput=False)
    out = nc.declare_dram_parameter("out", [QC, D], F32, isOutput=True)

    with tile.TileContext(nc, trace_sim=TRACE_SIM) as tc:
        with (
            tc.tile_pool(name="singles", bufs=1) as singles,
            tc.tile_pool(name="kqs", bufs=1) as kqs,
            tc.tile_pool(name="attnp", bufs=1) as attnp,
        ):
            ident_f = singles.tile([P, P], F32, tag="ident_f")
            make_identity(nc, ident_f)
            ident = singles.tile([P, P], F32R, tag="ident")
            nc.vector.tensor_copy(ident, ident_f)
            ones128 = singles.tile([P, 1], F32R, tag="ones128")
            nc.vector.memset(ones128.bitcast(F32), 1.0)
            ones_row = singles.tile([1, P], F32R, tag="ones_row")
            nc.vector.memset(ones_row.bitcast(F32), 1.0)
            ones512 = singles.tile([1, 512], F32R, tag="ones512")
            nc.vector.memset(ones512.bitcast(F32), 1.0)
            epst = singles.tile([1, 1], F32, tag="epst")
            nc.vector.memset(epst, EPS)
            bq_row = singles.tile([1, D], F32R, tag="bq_row")
            nc.sync.dma_start(out=bq_row, in_=bqp.rearrange("(o d) -> o d", o=1))

            qS = [kqs.tile([P, QC], F32R, tag=f"qS{m}", name=f"qS{m}")
                  for m in range(8)]
            attn = [attnp.tile([P, KV], F32R, tag=f"at{it}",
                               name=f"at{it}") for it in range(4)]

            # value pool opened early (addresses reserved below kTp) so vt
            # loads can run during S; DMAs are emitted late so they do not
            # head-of-line-block the queues at startup
            vp_cm = tc.tile_pool(name="vp", bufs=4)
            vp = vp_cm.__enter__()

            # qT first (it heads the longest dependency chain), then kT,
            # both split across the two HWDGE queues
            kTp_cm = tc.tile_pool(name="kTp", bufs=1)
            kTp = kTp_cm.__enter__()
            inq_cm = tc.tile_pool(name="inq", bufs=1)
            inq = inq_cm.__enter__()
            qtile = [inq.tile([P, QC], F32R, tag=f"qt{c}", name=f"qt{c}")
                     for c in range(8)]
            for c in range(8):
                nc.sync.dma_start(out=qtile[c], in_=qT[c * P:(c + 1) * P, :])
            ktile = [kTp.tile([P, KV], F32R, tag=f"kt{c}", name=f"kt{c}")
                     for c in range(8)]
            for c in (range(8) if PHASES != "pq" else ()):
                nc.sync.dma_start(out=ktile[c], in_=kT[c * P:(c + 1) * P, :])

            # ---------------- key side: R_k ----------------
            if True:
                with (
                    tc.tile_pool(name="sqk", bufs=2) as sqk,
                    tc.tile_pool(name="rowkp", bufs=1) as rowkp,
                ):
                    rowk = rowkp.tile([1, KV], F32R, tag="rowk")
                    rb_k = kqs.tile([P, KV], F32, tag="rbk")
                    with tc.tile_pool(name="pskrow", bufs=1, space="PSUM") as pskrow:
                        rowk_ps = pskrow.tile([1, KV], F32, tag="rowk_ps")
                        for c in (range(8) if PHASES != "pq" else ()):
                            sq = sqk.tile([P, KV], F32R, tag="sqk")
                            nc.vector.tensor_tensor(sq, ktile[c].bitcast(F32),
                                                    ktile[c].bitcast(F32),
                                                    ALU.mult)
                            for ns in range(4):
                                nc.tensor.matmul(
                                    rowk_ps[0:1, ns * 512:(ns + 1) * 512],
                                    lhsT=ones128,
                                    rhs=sq[:, ns * 512:(ns + 1) * 512],
                                    start=(c == 0), stop=(c == 7))
                        nc.vector.tensor_copy(rowk, rowk_ps)
                        nc.scalar.activation(rowk, rowk, AF.Ln, bias=epst,
                                             scale=1.0 / D)
                        nc.scalar.activation(rowk, rowk, AF.Exp, scale=-0.5)
                        rbk_ps = pskrow.tile([P, KV], F32, tag="rbk_ps")
                        for ns in range(4):
                            nc.tensor.matmul(
                                rbk_ps[:, ns * 512:(ns + 1) * 512],
                                lhsT=ones_row,
                                rhs=rowk[0:1, ns * 512:(ns + 1) * 512],
                                start=True, stop=True)
                        nc.vector.tensor_copy(rb_k, rbk_ps)

            # ---------------- query side: R_q + P_q ----------------
            with (
                tc.tile_pool(name="rbq_p", bufs=1) as rbq_p,
                tc.tile_pool(name="wqp", bufs=16) as wqp,
                tc.tile_pool(name="sqp", bufs=2) as sqp,
            ):
                rowq = rbq_p.tile([1, QC], F32R, tag="rowq")
                rb_q = rbq_p.tile([P, QC], F32, tag="rbq")
                with tc.tile_pool(name="psqrow", bufs=1, space="PSUM") as psqrow:
                    rowq_ps = psqrow.tile([1, QC], F32, tag="rowq_ps")
                    for c in range(8):
                        sq = sqp.tile([P, QC], F32R, tag="sq")
                        nc.vector.tensor_tensor(sq, qtile[c].bitcast(F32),
                                                qtile[c].bitcast(F32), ALU.mult)
                        nc.tensor.matmul(rowq_ps, lhsT=ones128, rhs=sq,
                                         start=(c == 0), stop=(c == 7))
                    nc.vector.tensor_copy(rowq, rowq_ps)
                    nc.scalar.activation(rowq, rowq, AF.Ln, bias=epst,
                                         scale=1.0 / D)
                    nc.scalar.activation(rowq, rowq, AF.Exp, scale=-0.5)
                    rbq_ps = psqrow.tile([P, QC], F32, tag="rbq_ps")
                    nc.tensor.matmul(rbq_ps, lhsT=ones_row, rhs=rowq,
                                     start=True, stop=True)
                    nc.vector.tensor_copy(rb_q, rbq_ps)
                # scale query columns by r_q in place (DVE/GPSIMD split)
                for c in range(8):
                    nc.vector.tensor_tensor(qtile[c], qtile[c].bitcast(F32),
                                            rb_q, ALU.mult)
                with tc.tile_pool(name="psq", bufs=2, space="PSUM") as psq:
                    for m in range(8):
                        wqs = []
                        for c in range(8):
                            wt = wqp.tile([P, P], F32R, tag="wq",
                                          name=f"wq{m}_{c}")
                            nc.sync.dma_start(
                                out=wt,
                                in_=wqT[c * P:(c + 1) * P, m * P:(m + 1) * P])
                            wqs.append(wt)
                        pqm = psq.tile([P, QC], F32, tag="pq", name=f"pq{m}")
                        for c in range(8):
                            nc.tensor.matmul(pqm, lhsT=wqs[c], rhs=qtile[c],
                                             start=(c == 0), stop=False)
                        nc.tensor.matmul(pqm,
                                         lhsT=bq_row[0:1, m * P:(m + 1) * P],
                                         rhs=ones512, start=False, stop=True)
                        nc.vector.tensor_copy(qS[m], pqm)

            inq_cm.__exit__(None, None, None)

            # -------- main loop: P_k(m) and S(heads 2m, 2m + 1) ---------
            # share one 8-bank PSUM pool ("big", [128,2048] tiles, bufs=2)
            with (
                tc.tile_pool(name="wkp", bufs=16) as wkp,
                tc.tile_pool(name="kSp", bufs=3) as kSp,
                tc.tile_pool(name="etp", bufs=3) as etp,
                tc.tile_pool(name="denp", bufs=16) as denp,
                tc.tile_pool(name="pkp", bufs=2, space="PSUM") as pkp,
                tc.tile_pool(name="ssp", bufs=2, space="PSUM") as ssp,
            ):
                for m in (range(8) if PHASES not in ("pq", "rk") else ()):
                    wts = []
                    for c in range(8):
                        wt = wkp.tile([P, P], F32R, tag="wk",
                                      name=f"wk{m}_{c}")
                        nc.sync.dma_start(
                            out=wt,
                            in_=wkT[c * P:(c + 1) * P, m * P:(m + 1) * P])
                        wts.append(wt)
                    kS = kSp.tile([P, KV], F32R, tag="kS", name=f"kS{m}")
                    for half in range(2):
                        pk = pkp.tile([P, 1024], F32, tag="pk",
                                      name=f"pk{m}_{half}")
                        for ns in range(2):
                            sl = slice(ns * 512, (ns + 1) * 512)
                            gl = slice(half * 1024 + ns * 512,
                                       half * 1024 + (ns + 1) * 512)
                            for c in range(8):
                                nc.tensor.matmul(
                                    pk[:, sl], lhsT=wts[c],
                                    rhs=ktile[c][:, gl],
                                    start=(c == 0), stop=(c == 7))
                        # kS = pk * r_k  (k bias dropped: it only shifts
                        # scores by a per-row constant, softmax-invariant)
                        nc.vector.scalar_tensor_tensor(
                            out=kS[:, half * 1024:(half + 1) * 1024],
                            in0=pk, scalar=1.0,
                            in1=rb_k[:, half * 1024:(half + 1) * 1024],
                            op0=ALU.mult, op1=ALU.mult)

                    if PHASES in ("pq", "rk", "pk"):
                        continue
                    for h in (2 * m, 2 * m + 1):
                        hp = (h % 2) * 64
                        for it in range(4):
                            i0 = it * P
                            et = etp.tile([P, KV], F32, tag="et")
                            dens = []
                            for half in range(2):
                                ps = ssp.tile([P, 1024], F32, tag="ss",
                                              name=f"s{h}_{it}_{half}")
                                for ns in range(2):
                                    gl = slice(half * 1024 + ns * 512,
                                               half * 1024 + (ns + 1) * 512)
                                    nc.tensor.matmul(
                                        ps[:, ns * 512:(ns + 1) * 512],
                                        lhsT=qS[m][hp:hp + 64, i0:i0 + P],
                                        rhs=kS[hp:hp + 64, gl],
                                        start=True, stop=True)
                                dh = denp.tile([P, 1], F32, tag="d")
                                nc.scalar.activation(
                                    et[:, half * 1024:(half + 1) * 1024],
                                    ps, AF.Exp, accum_out=dh)
                                dens.append(dh)
                            rd = denp.tile([P, 1], F32, tag="d")
                            nc.vector.tensor_tensor(dens[0], dens[0],
                                                    dens[1], ALU.add)
                            nc.vector.reciprocal(rd, dens[0])
                            if PHASES == "s_nocomb":
                                continue
                            if h == 0:
                                nc.vector.tensor_scalar_mul(attn[it], et, rd)
                            else:
                                nc.vector.scalar_tensor_tensor(
                                    out=attn[it], in0=et,
                                    scalar=rd, in1=attn[it],
                                    op0=ALU.mult, op1=ALU.add)

            kTp_cm.__exit__(None, None, None)

            vts = []
            for jc in range(16):
                vt = vp.tile([P, D], F32R, tag="vt", name=f"vt{jc}")
                nc.sync.dma_start(out=vt, in_=v[jc * P:(jc + 1) * P, :])
                vts.append(vt)

            # ---------------- T: transpose attn ----------------
            nd = DVE_COLS // P
            with tc.tile_pool(name="aTp", bufs=1) as aTp:
                with tc.tile_pool(name="pst", bufs=2, space="PSUM") as pst:
                    aT = []
                    for jc in (range(16) if PHASES in ("full", "tf") else ()):
                        tp = pst.tile([P, 512], F32R, tag="tp")
                        for it in range(4):
                            nc.tensor.transpose(tp[:, it * P:(it + 1) * P],
                                                attn[it][:, jc * P:(jc + 1) * P],
                                                ident)
                        a = aTp.tile([P, 512], F32R, tag=f"aT{jc}",
                                     name=f"aT{jc}")
                        nc.vector.tensor_copy(a, tp)
                        aT.append(a)

                # ---------------- F: features ----------------
                with (
                    tc.tile_pool(name="outp", bufs=2) as outp,
                    tc.tile_pool(name="psf", bufs=1, space="PSUM") as psf,
                ):
                    pf = [psf.tile([P, D], F32, tag=f"pf{it}", name=f"pf{it}")
                          for it in range(4)]
                    for jc in (range(16) if PHASES in ("full", "tf") else ()):
                        vt = vts[jc]
                        for it in range(4):
                            for eh in range(2):
                                nc.tensor.matmul(
                                    pf[it][:, eh * 512:(eh + 1) * 512],
                                    lhsT=aT[jc][:, it * P:(it + 1) * P],
                                    rhs=vt[:, eh * 512:(eh + 1) * 512],
                                    start=(jc == 0), stop=(jc == 15))
                    for it in (range(4) if PHASES in ("full", "tf") else ()):
                        o = outp.tile([P, D], F32, tag="o")
                        nc.vector.tensor_copy(o, pf[it])
                        nc.sync.dma_start(out=out[it * P:(it + 1) * P, :], in_=o)

            vp_cm.__exit__(None, None, None)

    nc.finalize()
    return nc


def _prep_in_maps(query, key, value, wq_norm, wk_norm, Wq, Wk, bq, bk):
    WqT = np.ascontiguousarray((Wq * wq_norm[None, :]).T, dtype=np.float32)
    WkT = np.ascontiguousarray((Wk * wk_norm[None, :]).T / np.float32(np.sqrt(DH)),
                               dtype=np.float32)
    vH = (value / np.float32(H)).astype(np.float32)
    in_maps = []
    for c in range(8):
        b, g = c // 4, c % 4
        in_maps.append({
            "qT": np.ascontiguousarray(query[b, g * QC:(g + 1) * QC, :].T),
            "kT": np.ascontiguousarray(key[b].T),
            "v": np.ascontiguousarray(vH[b]),
            "wqT": WqT,
            "wkT": WkT,
            "bq": np.ascontiguousarray(bq),
        })
    return in_maps


def kernel(query, key, value, wq_norm, wk_norm, Wq, Wk, bq, bk):
    query = np.asarray(query, dtype=np.float32)
    key = np.asarray(key, dtype=np.float32)
    value = np.asarray(value, dtype=np.float32)
    wq_norm = np.asarray(wq_norm, dtype=np.float32)
    wk_norm = np.asarray(wk_norm, dtype=np.float32)
    Wq = np.asarray(Wq, dtype=np.float32)
    Wk = np.asarray(Wk, dtype=np.float32)
    bq = np.asarray(bq, dtype=np.float32)
    bk = np.asarray(bk, dtype=np.float32)

    if "nc" not in _cache:
        _cache["nc"] = build_nc()
    nc = _cache["nc"]
    in_maps = _prep_in_maps(query, key, value, wq_norm, wk_norm, Wq, Wk, bq, bk)
    res = run_bass_kernel_spmd(nc, in_maps, list(range(8)))
    outp = np.zeros((B, Q, D), np.float32)
    for c in range(8):
        b, g = c // 4, c % 4
        outp[b, g * QC:(g + 1) * QC, :] = res.results[c]["out"]
    return outp

